# revision 17
# baseline (speedup 1.0000x reference)
"""Multi-head self-attention (B=4, T=2048, D=1024, H=16, Dh=64) on 8 trn2 cores.

Sharding: core c = (batch b = c//2, head-half = c%2). Each core computes the
attention output contribution of 8 heads for one batch element, including the
row-sharded output projection; the host sums the two half-partials per batch
and adds the folded bias (bv @ Wo + bo).

Per-core dataflow (bf16 matmuls, fp32 PSUM accumulation):
  xT [D,T] -> QT/KT [512,T] (scale 1/8 folded into Wq), V [T,512]
  per head pair (A,B) packed on PE row-groups:
    scoresT [k,q] = KT_h^T-slices x QT_h-slices  (K=64 contraction)
    E = exp(scoresT) on ScalarE (PSUM->SBUF, bf16)
    num = V_aug^T @ E accumulated over k tiles; V_aug carries a ones column
    so num row 64 (A) / 63 (B) is the softmax denominator.
  normalize: reciprocal_approx_fast on the rowsum rows, partition_broadcast,
  DVE multiply -> AOT [512,T] bf16; out = AOT^T-slices @ Wo -> [T,D] bf16.
"""

import numpy as np
import ml_dtypes

P = 128
T = 2048
D = 1024
DH = 512          # per-core head dims (8 heads x 64)
NK = D // P       # 8 contraction tiles for projections
NT = T // P       # 16 token tiles
QC = 512          # q-chunk width
NQC = T // QC     # 4
NPAIR = 4         # head pairs per core
BF = ml_dtypes.bfloat16

_CACHED_NC = None


def build_nc():
    global _CACHED_NC
    if _CACHED_NC is not None:
        return _CACHED_NC
    from contextlib import ExitStack
    import concourse.mybir as mybir
    import concourse.tile as tile
    from concourse import bacc
    from concourse.bass import ds

    f32 = mybir.dt.float32
    bf16 = mybir.dt.bfloat16
    EXP = mybir.ActivationFunctionType.Exp

    nc = bacc.Bacc("TRN2", target_bir_lowering=False, debug=False, num_devices=8)
    xt_d = nc.dram_tensor("xt", [D, T], bf16, kind="ExternalInput")
    wq_d = nc.dram_tensor("wq", [D, DH], bf16, kind="ExternalInput")
    wk_d = nc.dram_tensor("wk", [D, DH], bf16, kind="ExternalInput")
    wv_d = nc.dram_tensor("wv", [D, DH], bf16, kind="ExternalInput")
    wo_d = nc.dram_tensor("wo", [DH, D], bf16, kind="ExternalInput")
    bq_d = nc.dram_tensor("bq2", [1, DH], f32, kind="ExternalInput")
    bk_d = nc.dram_tensor("bk2", [1, DH], f32, kind="ExternalInput")
    out_d = nc.dram_tensor("out", [T, D], bf16, kind="ExternalOutput")

    with tile.TileContext(nc) as tc, ExitStack() as ctx:
        cpool = ctx.enter_context(tc.tile_pool(name="const", bufs=1))
        pp = ctx.enter_context(tc.tile_pool(name="proj", bufs=2, space="PSUM"))
        ps_s = ctx.enter_context(tc.tile_pool(name="scores", bufs=1, space="PSUM"))
        ps_n = ctx.enter_context(tc.tile_pool(name="num", bufs=1, space="PSUM"))
        epool = ctx.enter_context(tc.tile_pool(name="esb", bufs=2))
        rpool = ctx.enter_context(tc.tile_pool(name="rsb", bufs=2))
        opool = ctx.enter_context(tc.tile_pool(name="osb", bufs=3))

        xt = [cpool.tile([P, T], bf16, name=f"xt{k}", tag=f"xt{k}") for k in range(NK)]
        wq = cpool.tile([P, NK, DH], bf16, name="wq_s", tag="wq_s")
        wk = cpool.tile([P, NK, DH], bf16, name="wk_s", tag="wk_s")
        wv = cpool.tile([P, NK, DH], bf16, name="wv_s", tag="wv_s")
        wo = cpool.tile([P, NPAIR, D], bf16, name="wo_s", tag="wo_s")
        bq = cpool.tile([1, DH], f32, name="bq_s", tag="bq_s")
        bk = cpool.tile([1, DH], f32, name="bk_s", tag="bk_s")
        ones = cpool.tile([1, QC], f32, name="ones_s", tag="ones_s")
        qt = [cpool.tile([P, T], bf16, name=f"qt{r}", tag=f"qt{r}") for r in range(NPAIR)]
        kt = [cpool.tile([P, T], bf16, name=f"kt{r}", tag=f"kt{r}") for r in range(NPAIR)]
        # V layout per (token-tile, pair):
        #   [v_A(64), ones, zeros(32), ones, zeros(31), v_B(64)]
        # A lhsT = cols 0:65 -> num rows 0:63 = A, row 64 = rowsum_A
        # B lhsT = cols 65:193 -> num rows 64:127 = B, row 32 = rowsum_B
        # (rowsum rows sit on 32-aligned partitions for the DVE reciprocal)
        vsb = cpool.tile([P, NT, NPAIR, 193], bf16, name="v_s", tag="v_s")
        aot = [cpool.tile([P, T], bf16, name=f"aot{r}", tag=f"aot{r}") for r in range(NPAIR)]

        # loads
        nc.sync.dma_start(wq[:], wq_d[:].rearrange("(k p) n -> p k n", p=P))
        nc.sync.dma_start(bq[:], bq_d[:])
        for k in range(NK):
            nc.sync.dma_start(xt[k][:], xt_d[ds(k * P, P), :])
        nc.sync.dma_start(wk[:], wk_d[:].rearrange("(k p) n -> p k n", p=P))
        nc.sync.dma_start(bk[:], bk_d[:])
        nc.sync.dma_start(wv[:], wv_d[:].rearrange("(k p) n -> p k n", p=P))
        nc.sync.dma_start(wo[:], wo_d[:].rearrange("(r p) n -> p r n", p=P))
        nc.vector.memset(ones[:], 1.0)
        nc.gpsimd.memset(vsb[:, :, :, 64:65], 1.0)
        nc.gpsimd.memset(vsb[:, :, :, 65:97], 0.0)
        nc.gpsimd.memset(vsb[:, :, :, 97:98], 1.0)
        nc.gpsimd.memset(vsb[:, :, :, 98:129], 0.0)

        def proj_qk(dst, w, b, r, qc):
            ps = pp.tile([P, QC], f32, name="ps_p", tag="ps_p")
            for k in range(NK):
                nc.tensor.matmul(ps[:], w[:, k, ds(r * P, P)], xt[k][:, ds(qc * QC, QC)],
                                 start=(k == 0), stop=False)
            # bias folded in as a rank-1 fp32 matmul: ps += b[r-slice]^T @ ones
            nc.tensor.matmul(ps[:], b[:, ds(r * P, P)], ones[:],
                             start=False, stop=True)
            nc.vector.tensor_copy(dst[:, ds(qc * QC, QC)], ps[:])

        def proj_v(t):
            ps = pp.tile([P, DH], f32, name="ps_p", tag="ps_p")
            for k in range(NK):
                nc.tensor.matmul(ps[:], xt[k][:, ds(t * P, P)], wv[:, k, :],
                                 start=(k == 0), stop=(k == NK - 1))
            psv = ps.rearrange("p (r hd) -> p r hd", r=NPAIR)
            nc.vector.tensor_copy(vsb[:, t, :, 0:64], psv[:, :, 0:64])
            nc.vector.tensor_copy(vsb[:, t, :, 129:193], psv[:, :, 64:128])

        # pair-0 Q/K projections, then V
        for qc in range(NQC):
            proj_qk(qt[0], wq, bq, 0, qc)
            proj_qk(kt[0], wk, bk, 0, qc)
        for t in range(NT):
            proj_v(t)

        # attention; Q/K projections for pair r+1 are interleaved into the
        # group loop so the PE fills ACT-bound gaps with projection work.
        for r in range(NPAIR):
            later = []
            if r + 1 < NPAIR:
                later = [(qt[r + 1], wq, bq) , (kt[r + 1], wk, bk)]
            for qc in range(NQC):
                nA = ps_n.tile([65, QC], f32, name="nA", tag="nA")
                nB = ps_n.tile([P, QC], f32, name="nB", tag="nB")
                for g in range(NT // 2):
                    t0 = 2 * g
                    sA = ps_s.tile([P, 2, QC], f32, name="sA", tag="sA")
                    sB = ps_s.tile([P, 2, QC], f32, name="sB", tag="sB")
                    for i in range(2):
                        t = t0 + i
                        nc.tensor.matmul(sA[:, i, :], kt[r][0:64, ds(t * P, P)],
                                         qt[r][0:64, ds(qc * QC, QC)], start=True, stop=True)
                        nc.tensor.matmul(sB[:, i, :], kt[r][64:P, ds(t * P, P)],
                                         qt[r][64:P, ds(qc * QC, QC)], start=True, stop=True)
                    eA = epool.tile([P, 2, QC], bf16, name="eA", tag="eA")
                    eB = epool.tile([P, 2, QC], bf16, name="eB", tag="eB")
                    nc.scalar.activation(eA[:], sA[:], EXP)
                    nc.scalar.activation(eB[:], sB[:], EXP)
                    for i in range(2):
                        t = t0 + i
                        first = (g == 0 and i == 0)
                        last = (g == NT // 2 - 1 and i == 1)
                        nc.tensor.matmul(nA[:], vsb[:, t, r, 0:65], eA[:, i, :],
                                         start=first, stop=last)
                        nc.tensor.matmul(nB[:], vsb[:, t, r, 65:193], eB[:, i, :],
                                         start=first, stop=last)
                # rowsum rows (PSUM p64 / p32) -> partition 0 (32-aligned DVE
                # shift-copies), fast reciprocal at base 0, broadcast across
                # partitions via rank-1 fp32 matmul, normalize.
                r2a = rpool.tile([1, QC], f32, name="r2a", tag="r2a")
                r2b = rpool.tile([1, QC], f32, name="r2b", tag="r2b")
                nc.vector.tensor_copy(r2a[:], nA[64:65, :])
                nc.vector.tensor_copy(r2b[:], nB[32:33, :])
                r3a = rpool.tile([1, QC], f32, name="r3a", tag="r3a")
                r3b = rpool.tile([1, QC], f32, name="r3b", tag="r3b")
                nc.vector.reciprocal_approx_fast(r3a[:], r2a[:])
                nc.vector.reciprocal_approx_fast(r3b[:], r2b[:])
                rbc1 = pp.tile([64, QC], f32, name="rbc1", tag="ps_p")
                rbc2 = pp.tile([64, QC], f32, name="rbc2", tag="ps_p")
                nc.tensor.matmul(rbc1[:], ones[0:1, 0:64], r3a[:], start=True, stop=True)
                nc.tensor.matmul(rbc2[:], ones[0:1, 0:64], r3b[:], start=True, stop=True)
                rbc = rpool.tile([P, QC], f32, name="rbc", tag="rbc")
                nc.vector.tensor_copy(rbc[0:64, :], rbc1[:])
                nc.vector.tensor_copy(rbc[64:P, :], rbc2[0:64, :])
                nc.vector.tensor_mul(aot[r][0:64, ds(qc * QC, QC)], nA[0:64, :], rbc[0:64, :])
                nc.vector.tensor_mul(aot[r][64:P, ds(qc * QC, QC)], nB[64:P, :], rbc[64:P, :])
                # two interleaved projection chains for the next pair
                if later:
                    proj_qk(later[0][0], later[0][1], later[0][2], r + 1, qc)
                    proj_qk(later[1][0], later[1][1], later[1][2], r + 1, qc)

        # output projection: out[q, :] accumulated over the 4 pair d-chunks
        for j in range(NT):
            for n in range(D // QC):
                ps = pp.tile([P, QC], f32, name="ps_p", tag="ps_p")
                for r in range(NPAIR):
                    nc.tensor.matmul(ps[:], aot[r][:, ds(j * P, P)], wo[:, r, ds(n * QC, QC)],
                                     start=(r == 0), stop=(r == NPAIR - 1))
                o = opool.tile([P, QC], bf16, name="ost", tag="ost")
                nc.vector.tensor_copy(o[:], ps[:])
                nc.sync.dma_start(out_d[ds(j * P, P), ds(n * QC, QC)], o[:])

    nc.compile()

    _CACHED_NC = nc
    return nc


def prepare_in_maps(inputs):
    x = np.asarray(inputs["x"], np.float32)
    Wq = np.asarray(inputs["Wq"], np.float32)
    bq = np.asarray(inputs["bq"], np.float32)
    Wk = np.asarray(inputs["Wk"], np.float32)
    bk = np.asarray(inputs["bk"], np.float32)
    Wv = np.asarray(inputs["Wv"], np.float32)
    in_maps = []
    for c in range(8):
        b, half = c // 2, c % 2
        cols = slice(half * DH, (half + 1) * DH)
        bqs = (bq[cols] / 8.0).astype(np.float32)
        bks = bk[cols].astype(np.float32)
        in_maps.append({
            "xt": np.ascontiguousarray(x[b].T).astype(BF),
            "wq": np.ascontiguousarray(Wq[:, cols] / 8.0).astype(BF),
            "wk": np.ascontiguousarray(Wk[:, cols]).astype(BF),
            "wv": np.ascontiguousarray(np.asarray(inputs["Wv"], np.float32)[:, cols]).astype(BF),
            "wo": np.ascontiguousarray(np.asarray(inputs["Wo"], np.float32)[cols, :]).astype(BF),
            "bq2": bqs.reshape(1, DH).copy(),
            "bk2": bks.reshape(1, DH).copy(),
        })
    return in_maps


def postprocess(results, inputs):
    bv = np.asarray(inputs["bv"], np.float64)
    Wo = np.asarray(inputs["Wo"], np.float64)
    bo = np.asarray(inputs["bo"], np.float64)
    bo_eff = (bv @ Wo + bo).astype(np.float32)
    out = np.empty((4, T, D), np.float32)
    for b in range(4):
        out[b] = (results[2 * b]["out"].astype(np.float32)
                  + results[2 * b + 1]["out"].astype(np.float32)
                  + bo_eff[None, :])
    return out


def kernel(**inputs):
    from concourse.bass_utils import run_bass_kernel_spmd
    nc = build_nc()
    in_maps = prepare_in_maps(inputs)
    res = run_bass_kernel_spmd(nc, in_maps, core_ids=list(range(8)))
    return postprocess(res.results, inputs)


# revision 19
# speedup vs baseline: 1.3592x; 1.3592x over previous
"""Multi-head self-attention (B=4, T=2048, D=1024, H=16, Dh=64) on 8 trn2 cores.

Sharding: core c = (batch b = c//2, head-half = c%2). Each core computes the
attention output contribution of 8 heads for one batch element, including the
row-sharded output projection; the host sums the two half-partials per batch
and adds the folded bias (bv @ Wo + bo).

Per-core dataflow (bf16 matmuls, fp32 PSUM accumulation):
  xT [D,T] -> QT/KT [512,T] (scale 1/8 folded into Wq), V [T,512]
  per head pair (A,B), per 512-wide q chunk, per k token-tile t:
    scoresT[:,A/B] = KT-slice^T x QT-slice   (K=64, A/B packed on PE row groups)
    E = exp(scoresT) on ScalarE ([128,1024] PSUM->SBUF bf16, one op per t)
    num_A/B += V_aug[t]^T @ E  (V carries a ones column -> rowsum lands in
    num row 64 (A) / 32 (B))
  softmax denominator: copy nums PSUM->SBUF (releases banks), shift rowsum
  rows to partition 0, reciprocal_approx_fast, broadcast via rank-1 fp32
  matmuls, multiply -> AOT [512,T] bf16;  out = AOT^T-slices @ Wo [T,D] bf16.

Emission is software-pipelined: projections (V, next-pair QT/KT) and the
output projection of finished q-chunks are sprinkled into the per-k-tile
slots of the attention loop so the PE fills the gaps of the ACT-bound
steady state; the normalize chain of chunk i is emitted at the top of
chunk i+1 so it never blocks the in-order PE queue.
"""

import numpy as np
import ml_dtypes

P = 128
T = 2048
D = 1024
DH = 512          # per-core head dims (8 heads x 64)
NK = D // P       # 8 contraction tiles for projections
NT = T // P       # 16 token tiles
QC = 512          # q-chunk width
NQC = T // QC     # 4
NPAIR = 4         # head pairs per core
BF = ml_dtypes.bfloat16

_CACHED_NC = None


def build_nc():
    global _CACHED_NC
    if _CACHED_NC is not None:
        return _CACHED_NC
    from contextlib import ExitStack
    import concourse.mybir as mybir
    import concourse.tile as tile
    from concourse import bacc
    from concourse.bass import ds

    f32 = mybir.dt.float32
    bf16 = mybir.dt.bfloat16
    EXP = mybir.ActivationFunctionType.Exp

    nc = bacc.Bacc("TRN2", target_bir_lowering=False, debug=False, num_devices=8)
    xt_d = nc.dram_tensor("xt", [D, T], bf16, kind="ExternalInput")
    wq_d = nc.dram_tensor("wq", [D, DH], bf16, kind="ExternalInput")
    wk_d = nc.dram_tensor("wk", [D, DH], bf16, kind="ExternalInput")
    wv_d = nc.dram_tensor("wv", [D, DH], bf16, kind="ExternalInput")
    wo_d = nc.dram_tensor("wo", [DH, D], bf16, kind="ExternalInput")
    bq_d = nc.dram_tensor("bq2", [1, DH], f32, kind="ExternalInput")
    bk_d = nc.dram_tensor("bk2", [1, DH], f32, kind="ExternalInput")
    out_d = nc.dram_tensor("out", [T, D], bf16, kind="ExternalOutput")

    with tile.TileContext(nc) as tc, ExitStack() as ctx:
        cpool = ctx.enter_context(tc.tile_pool(name="const", bufs=1))
        pp = ctx.enter_context(tc.tile_pool(name="proj", bufs=2, space="PSUM"))
        ps_s = ctx.enter_context(tc.tile_pool(name="scores", bufs=2, space="PSUM"))
        ps_n = ctx.enter_context(tc.tile_pool(name="num", bufs=1, space="PSUM"))
        epool = ctx.enter_context(tc.tile_pool(name="esb", bufs=3))
        npool = ctx.enter_context(tc.tile_pool(name="nsb", bufs=2))
        rpool = ctx.enter_context(tc.tile_pool(name="rsb", bufs=2))
        opool = ctx.enter_context(tc.tile_pool(name="osb", bufs=3))

        xt = [cpool.tile([P, T], bf16, name=f"xt{k}", tag=f"xt{k}") for k in range(NK)]
        wq = cpool.tile([P, NK, DH], bf16, name="wq_s", tag="wq_s")
        wk = cpool.tile([P, NK, DH], bf16, name="wk_s", tag="wk_s")
        wv = cpool.tile([P, NK, DH], bf16, name="wv_s", tag="wv_s")
        wo = cpool.tile([P, NPAIR, D], bf16, name="wo_s", tag="wo_s")
        bq = cpool.tile([1, DH], f32, name="bq_s", tag="bq_s")
        bk = cpool.tile([1, DH], f32, name="bk_s", tag="bk_s")
        ones = cpool.tile([1, QC], f32, name="ones_s", tag="ones_s")
        qt = [cpool.tile([P, T], bf16, name=f"qt{r}", tag=f"qt{r}") for r in range(NPAIR)]
        kt = [cpool.tile([P, T], bf16, name=f"kt{r}", tag=f"kt{r}") for r in range(NPAIR)]
        # V layout per (token-tile, pair):
        #   [v_A(64), ones, zeros(32), ones, zeros(31), v_B(64)]
        # A lhsT = cols 0:65 -> num rows 0:63 = A, row 64 = rowsum_A
        # B lhsT = cols 65:193 -> num rows 64:127 = B, row 32 = rowsum_B
        vsb = cpool.tile([P, NT, NPAIR, 193], bf16, name="v_s", tag="v_s")
        aot = [cpool.tile([P, T], bf16, name=f"aot{r}", tag=f"aot{r}") for r in range(NPAIR)]

        # loads
        nc.sync.dma_start(wq[:], wq_d[:].rearrange("(k p) n -> p k n", p=P))
        nc.sync.dma_start(bq[:], bq_d[:])
        nc.sync.dma_start(wk[:], wk_d[:].rearrange("(k p) n -> p k n", p=P))
        nc.sync.dma_start(bk[:], bk_d[:])
        for k in range(NK):
            nc.sync.dma_start(xt[k][:], xt_d[ds(k * P, P), :])
        nc.sync.dma_start(wv[:], wv_d[:].rearrange("(k p) n -> p k n", p=P))
        nc.sync.dma_start(wo[:], wo_d[:].rearrange("(r p) n -> p r n", p=P))
        nc.vector.memset(ones[:], 1.0)
        nc.gpsimd.memset(vsb[:, :, :, 64:65], 1.0)
        nc.gpsimd.memset(vsb[:, :, :, 65:97], 0.0)
        nc.gpsimd.memset(vsb[:, :, :, 97:98], 1.0)
        nc.gpsimd.memset(vsb[:, :, :, 98:129], 0.0)

        def proj_qk(dst, w, b, r, qc):
            ps = pp.tile([P, QC], f32, name="ps_p", tag="ps_p")
            for k in range(NK):
                nc.tensor.matmul(ps[:], w[:, k, ds(r * P, P)], xt[k][:, ds(qc * QC, QC)],
                                 start=(k == 0), stop=False)
            nc.tensor.matmul(ps[:], b[:, ds(r * P, P)], ones[:],
                             start=False, stop=True)
            nc.vector.tensor_copy(dst[:, ds(qc * QC, QC)], ps[:])

        def proj_v(t):
            ps = pp.tile([P, DH], f32, name="ps_p", tag="ps_p")
            for k in range(NK):
                nc.tensor.matmul(ps[:], xt[k][:, ds(t * P, P)], wv[:, k, :],
                                 start=(k == 0), stop=(k == NK - 1))
            psv = ps.rearrange("p (r hd) -> p r hd", r=NPAIR)
            nc.vector.tensor_copy(vsb[:, t, :, 0:64], psv[:, :, 0:64])
            nc.vector.tensor_copy(vsb[:, t, :, 129:193], psv[:, :, 64:128])

        def proj_out(j, n):
            ps = pp.tile([P, QC], f32, name="ps_p", tag="ps_p")
            for r in range(NPAIR):
                nc.tensor.matmul(ps[:], aot[r][:, ds(j * P, P)], wo[:, r, ds(n * QC, QC)],
                                 start=(r == 0), stop=(r == NPAIR - 1))
            o = opool.tile([P, QC], bf16, name="ost", tag="ost")
            nc.vector.tensor_copy(o[:], ps[:])
            nc.sync.dma_start(out_d[ds(j * P, P), ds(n * QC, QC)], o[:])

        def emit_normalize(st):
            nA, nB, r, qc = st
            # PSUM -> SBUF copies release the num banks promptly; the rest of
            # the chain runs off the critical path.
            cA = npool.tile([65, QC], f32, name="cA", tag="cA")
            cB = npool.tile([P, QC], f32, name="cB", tag="cB")
            nc.vector.tensor_copy(cA[:], nA[:])
            nc.vector.tensor_copy(cB[:], nB[:])
            r2a = rpool.tile([1, QC], f32, name="r2a", tag="r2a")
            r2b = rpool.tile([1, QC], f32, name="r2b", tag="r2b")
            nc.vector.tensor_copy(r2a[:], cA[64:65, :])
            nc.vector.tensor_copy(r2b[:], cB[32:33, :])
            r3a = rpool.tile([1, QC], f32, name="r3a", tag="r3a")
            r3b = rpool.tile([1, QC], f32, name="r3b", tag="r3b")
            nc.vector.reciprocal_approx_fast(r3a[:], r2a[:])
            nc.vector.reciprocal_approx_fast(r3b[:], r2b[:])
            rbc1 = pp.tile([64, QC], f32, name="rbc1", tag="ps_p")
            rbc2 = pp.tile([64, QC], f32, name="rbc2", tag="ps_p")
            nc.tensor.matmul(rbc1[:], ones[0:1, 0:64], r3a[:], start=True, stop=True)
            nc.tensor.matmul(rbc2[:], ones[0:1, 0:64], r3b[:], start=True, stop=True)
            rbc = rpool.tile([P, QC], f32, name="rbc", tag="rbc")
            nc.vector.tensor_copy(rbc[0:64, :], rbc1[:])
            nc.vector.tensor_copy(rbc[64:P, :], rbc2[0:64, :])
            nc.vector.tensor_mul(aot[r][0:64, ds(qc * QC, QC)], cA[0:64, :], rbc[0:64, :])
            nc.vector.tensor_mul(aot[r][64:P, ds(qc * QC, QC)], cB[64:P, :], rbc[64:P, :])

        # startup: KT spans all k tokens, so all 4 KT-pair0 chains (plus the
        # first QT chunk and the first two V token-tiles) precede attention.
        proj_qk(qt[0], wq, bq, 0, 0)
        for q2 in range(NQC):
            proj_qk(kt[0], wk, bk, 0, q2)
        proj_v(0)
        proj_v(1)

        pending_norm = None
        for r in range(NPAIR):
            for qc in range(NQC):
                # sprinkle work for this (r, qc) window, keyed by slot t
                sprinkle = {}
                if r == 0 and qc == 0:
                    for t in range(2, NT):
                        sprinkle.setdefault(t - 2, []).append(("v", t))
                    for i, q2 in enumerate((1, 2, 3)):
                        sprinkle.setdefault(3 + 4 * i, []).append(("q", 0, q2))
                if r < NPAIR - 1:
                    # next pair's projections: KT chains first (needed from
                    # t=0 of its qc0), QT chains later
                    kind, c0 = (("k", 2 * qc) if qc < 2 else ("q", 2 * (qc - 2)))
                    sprinkle.setdefault(4, []).append((kind, r + 1, c0))
                    sprinkle.setdefault(10, []).append((kind, r + 1, c0 + 1))
                if r == NPAIR - 1 and qc > 0:
                    for i in range(8):
                        j = (qc - 1) * NPAIR + i // 2
                        sprinkle.setdefault(2 * i + 1, []).append(("o", j, i % 2))

                nA = ps_n.tile([65, QC], f32, name="nA", tag="nA")
                nB = ps_n.tile([P, QC], f32, name="nB", tag="nB")
                es = {}
                for t in range(NT):
                    sc = ps_s.tile([P, 2, QC], f32, name="sc", tag="sc")
                    nc.tensor.matmul(sc[:, 0, :], kt[r][0:64, ds(t * P, P)],
                                     qt[r][0:64, ds(qc * QC, QC)], start=True, stop=True)
                    nc.tensor.matmul(sc[:, 1, :], kt[r][64:P, ds(t * P, P)],
                                     qt[r][64:P, ds(qc * QC, QC)], start=True, stop=True)
                    e = epool.tile([P, 2, QC], bf16, name="eT", tag="eT")
                    nc.scalar.activation(e[:], sc[:], EXP)
                    es[t] = e
                    if t == 0 and pending_norm is not None:
                        emit_normalize(pending_norm)
                        pending_norm = None
                    if t >= 1:
                        tp = t - 1
                        nc.tensor.matmul(nA[:], vsb[:, tp, r, 0:65], es[tp][:, 0, :],
                                         start=(tp == 0), stop=False)
                        nc.tensor.matmul(nB[:], vsb[:, tp, r, 65:193], es[tp][:, 1, :],
                                         start=(tp == 0), stop=False)
                        del es[tp]
                    for item in sprinkle.get(t, ()):
                        if item[0] == "v":
                            proj_v(item[1])
                        elif item[0] == "q":
                            proj_qk(qt[item[1]], wq, bq, item[1], item[2])
                        elif item[0] == "k":
                            proj_qk(kt[item[1]], wk, bk, item[1], item[2])
                        elif item[0] == "o":
                            proj_out(item[1], item[2])
                tp = NT - 1
                nc.tensor.matmul(nA[:], vsb[:, tp, r, 0:65], es[tp][:, 0, :],
                                 start=False, stop=True)
                nc.tensor.matmul(nB[:], vsb[:, tp, r, 65:193], es[tp][:, 1, :],
                                 start=False, stop=True)
                del es[tp]
                pending_norm = (nA, nB, r, qc)

        emit_normalize(pending_norm)
        # tail: output projection of the last q-chunk
        for j in range((NQC - 1) * NPAIR, NT):
            for n in range(D // QC):
                proj_out(j, n)

    nc.compile()

    _CACHED_NC = nc
    return nc


def prepare_in_maps(inputs):
    x = np.asarray(inputs["x"], np.float32)
    Wq = np.asarray(inputs["Wq"], np.float32)
    bq = np.asarray(inputs["bq"], np.float32)
    Wk = np.asarray(inputs["Wk"], np.float32)
    bk = np.asarray(inputs["bk"], np.float32)
    Wv = np.asarray(inputs["Wv"], np.float32)
    Wo = np.asarray(inputs["Wo"], np.float32)
    in_maps = []
    for c in range(8):
        b, half = c // 2, c % 2
        cols = slice(half * DH, (half + 1) * DH)
        in_maps.append({
            "xt": np.ascontiguousarray(x[b].T).astype(BF),
            "wq": np.ascontiguousarray(Wq[:, cols] / 8.0).astype(BF),
            "wk": np.ascontiguousarray(Wk[:, cols]).astype(BF),
            "wv": np.ascontiguousarray(Wv[:, cols]).astype(BF),
            "wo": np.ascontiguousarray(Wo[cols, :]).astype(BF),
            "bq2": (bq[cols] / 8.0).astype(np.float32).reshape(1, DH).copy(),
            "bk2": bk[cols].astype(np.float32).reshape(1, DH).copy(),
        })
    return in_maps


def postprocess(results, inputs):
    bv = np.asarray(inputs["bv"], np.float64)
    Wo = np.asarray(inputs["Wo"], np.float64)
    bo = np.asarray(inputs["bo"], np.float64)
    bo_eff = (bv @ Wo + bo).astype(np.float32)
    out = np.empty((4, T, D), np.float32)
    for b in range(4):
        out[b] = (results[2 * b]["out"].astype(np.float32)
                  + results[2 * b + 1]["out"].astype(np.float32)
                  + bo_eff[None, :])
    return out


def kernel(**inputs):
    from concourse.bass_utils import run_bass_kernel_spmd
    nc = build_nc()
    in_maps = prepare_in_maps(inputs)
    res = run_bass_kernel_spmd(nc, in_maps, core_ids=list(range(8)))
    return postprocess(res.results, inputs)


# revision 26
# speedup vs baseline: 1.4248x; 1.0482x over previous
"""Multi-head self-attention (B=4, T=2048, D=1024, H=16, Dh=64) on 8 trn2 cores.

Sharding: core c = (batch b = c//2, head-half = c%2). Each core computes the
attention output contribution of 8 heads for one batch element, including the
row-sharded output projection; the host sums the two half-partials per batch
and adds the folded bias (bv @ Wo + bo).

Per-core dataflow (bf16 matmuls, fp32 PSUM accumulation):
  xT [D,T] -> QT/KT [512,T] (scale 1/8 folded into Wq), V [T,512]
  per head pair (A,B), per 512-wide q chunk, per k token-tile t:
    scoresT[:,A/B] = KT-slice^T x QT-slice   (K=64, A/B packed on PE row groups)
    E = exp(scoresT) on ScalarE ([128,1024] PSUM->SBUF bf16, one op per t)
    num_A/B += V_aug[t]^T @ E  (V carries a ones column -> rowsum lands in
    num row 64 (A) / 32 (B))
  softmax denominator: copy nums PSUM->SBUF (releases banks), shift rowsum
  rows to partition 0, reciprocal_approx_fast, broadcast via rank-1 fp32
  matmuls, multiply -> AOT [512,T] bf16;  out = AOT^T-slices @ Wo [T,D] bf16.

Emission is software-pipelined: projections (V, next-pair QT/KT) and the
output projection of finished q-chunks are sprinkled into the per-k-tile
slots of the attention loop so the PE fills the gaps of the ACT-bound
steady state; the normalize chain of chunk i is emitted at the top of
chunk i+1 so it never blocks the in-order PE queue.
"""

import numpy as np
import ml_dtypes

P = 128
T = 2048
D = 1024
DH = 512          # per-core head dims (8 heads x 64)
NK = D // P       # 8 contraction tiles for projections
NT = T // P       # 16 token tiles
QC = 512          # q-chunk width
NQC = T // QC     # 4
NPAIR = 4         # head pairs per core
BF = ml_dtypes.bfloat16

_CACHED_NC = None


def build_nc():
    global _CACHED_NC
    if _CACHED_NC is not None:
        return _CACHED_NC
    from contextlib import ExitStack
    import concourse.mybir as mybir
    import concourse.tile as tile
    from concourse import bacc
    from concourse.bass import ds

    f32 = mybir.dt.float32
    bf16 = mybir.dt.bfloat16
    EXP = mybir.ActivationFunctionType.Exp

    nc = bacc.Bacc("TRN2", target_bir_lowering=False, debug=False, num_devices=8)
    xt_d = nc.dram_tensor("xt", [D, T], bf16, kind="ExternalInput")
    wq_d = nc.dram_tensor("wq", [D, DH], bf16, kind="ExternalInput")
    wk_d = nc.dram_tensor("wk", [D, DH], bf16, kind="ExternalInput")
    wv_d = nc.dram_tensor("wv", [D, DH], bf16, kind="ExternalInput")
    wo_d = nc.dram_tensor("wo", [DH, D], bf16, kind="ExternalInput")
    bq_d = nc.dram_tensor("bq2", [P, NPAIR], f32, kind="ExternalInput")
    bk_d = nc.dram_tensor("bk2", [P, NPAIR], f32, kind="ExternalInput")
    out_d = nc.dram_tensor("out", [T, D], bf16, kind="ExternalOutput")

    with tile.TileContext(nc) as tc, ExitStack() as ctx:
        cpool = ctx.enter_context(tc.tile_pool(name="const", bufs=1))
        pp = ctx.enter_context(tc.tile_pool(name="proj", bufs=2, space="PSUM"))
        ps_s = ctx.enter_context(tc.tile_pool(name="scores", bufs=2, space="PSUM"))
        ps_n = ctx.enter_context(tc.tile_pool(name="num", bufs=1, space="PSUM"))
        epool = ctx.enter_context(tc.tile_pool(name="esb", bufs=4))
        npool = ctx.enter_context(tc.tile_pool(name="nsb", bufs=2))
        rpool = ctx.enter_context(tc.tile_pool(name="rsb", bufs=2))
        opool = ctx.enter_context(tc.tile_pool(name="osb", bufs=3))

        xt = [cpool.tile([P, T], bf16, name=f"xt{k}", tag=f"xt{k}") for k in range(NK)]
        wq = cpool.tile([P, NK, DH], bf16, name="wq_s", tag="wq_s")
        wk = cpool.tile([P, NK, DH], bf16, name="wk_s", tag="wk_s")
        wv = cpool.tile([P, NK, DH], bf16, name="wv_s", tag="wv_s")
        wo = cpool.tile([P, NPAIR, D], bf16, name="wo_s", tag="wo_s")
        bq = cpool.tile([P, NPAIR], f32, name="bq_s", tag="bq_s")
        bk = cpool.tile([P, NPAIR], f32, name="bk_s", tag="bk_s")
        ones = cpool.tile([1, QC], f32, name="ones_s", tag="ones_s")
        qt = [cpool.tile([P, T], bf16, name=f"qt{r}", tag=f"qt{r}") for r in range(NPAIR)]
        kt = [cpool.tile([P, T], bf16, name=f"kt{r}", tag=f"kt{r}") for r in range(NPAIR)]
        # V layout per (token-tile, pair):
        #   [v_A(64), ones, zeros(32), ones, zeros(31), v_B(64)]
        # A lhsT = cols 0:65 -> num rows 0:63 = A, row 64 = rowsum_A
        # B lhsT = cols 65:193 -> num rows 64:127 = B, row 32 = rowsum_B
        vsb = cpool.tile([P, NT, NPAIR, 193], bf16, name="v_s", tag="v_s")
        aot = [cpool.tile([P, T], bf16, name=f"aot{r}", tag=f"aot{r}") for r in range(NPAIR)]

        # loads
        nc.sync.dma_start(wq[:], wq_d[:].rearrange("(k p) n -> p k n", p=P))
        nc.sync.dma_start(bq[:], bq_d[:])
        nc.sync.dma_start(wk[:], wk_d[:].rearrange("(k p) n -> p k n", p=P))
        nc.sync.dma_start(bk[:], bk_d[:])
        for k in range(NK):
            nc.sync.dma_start(xt[k][:], xt_d[ds(k * P, P), :])
        nc.sync.dma_start(wv[:], wv_d[:].rearrange("(k p) n -> p k n", p=P))
        nc.sync.dma_start(wo[:], wo_d[:].rearrange("(r p) n -> p r n", p=P))
        nc.vector.memset(ones[:], 1.0)
        nc.gpsimd.memset(vsb[:, :, :, 64:65], 1.0)
        nc.gpsimd.memset(vsb[:, :, :, 65:97], 0.0)
        nc.gpsimd.memset(vsb[:, :, :, 97:98], 1.0)
        nc.gpsimd.memset(vsb[:, :, :, 98:129], 0.0)

        # projection chains are emitted in two halves so a sprinkled chain
        # never inserts more than ~1us of PE work between attention slots
        def proj_qk_a(dst, w, b, r, qc):
            ps = pp.tile([P, QC], f32, name="ps_p", tag="ps_p")
            for k in range(NK // 2):
                nc.tensor.matmul(ps[:], w[:, k, ds(r * P, P)], xt[k][:, ds(qc * QC, QC)],
                                 start=(k == 0), stop=False)
            return ps

        def proj_qk_b(ps, dst, w, b, r, qc):
            for k in range(NK // 2, NK):
                nc.tensor.matmul(ps[:], w[:, k, ds(r * P, P)], xt[k][:, ds(qc * QC, QC)],
                                 start=False, stop=(k == NK - 1))
            nc.vector.tensor_scalar_add(dst[:, ds(qc * QC, QC)], ps[:], b[:, r:r + 1])

        def proj_qk(dst, w, b, r, qc):
            proj_qk_b(proj_qk_a(dst, w, b, r, qc), dst, w, b, r, qc)

        def proj_v_a(t):
            ps = pp.tile([P, DH], f32, name="ps_p", tag="ps_p")
            for k in range(NK // 2):
                nc.tensor.matmul(ps[:], xt[k][:, ds(t * P, P)], wv[:, k, :],
                                 start=(k == 0), stop=False)
            return ps

        def proj_v_b(ps, t):
            for k in range(NK // 2, NK):
                nc.tensor.matmul(ps[:], xt[k][:, ds(t * P, P)], wv[:, k, :],
                                 start=False, stop=(k == NK - 1))
            psv = ps.rearrange("p (r hd) -> p r hd", r=NPAIR)
            nc.vector.tensor_copy(vsb[:, t, :, 0:64], psv[:, :, 0:64])
            nc.vector.tensor_copy(vsb[:, t, :, 129:193], psv[:, :, 64:128])

        def proj_v(t):
            proj_v_b(proj_v_a(t), t)

        def proj_out(j, n):
            ps = pp.tile([P, QC], f32, name="ps_p", tag="ps_p")
            for r in range(NPAIR):
                nc.tensor.matmul(ps[:], aot[r][:, ds(j * P, P)], wo[:, r, ds(n * QC, QC)],
                                 start=(r == 0), stop=(r == NPAIR - 1))
            o = opool.tile([P, QC], bf16, name="ost", tag="ost")
            nc.vector.tensor_copy(o[:], ps[:])
            nc.sync.dma_start(out_d[ds(j * P, P), ds(n * QC, QC)], o[:])

        def emit_normalize(st):
            nA, nB, r, qc = st
            # PSUM -> SBUF copies release the num banks promptly; the rest of
            # the chain runs off the critical path.
            cA = npool.tile([65, QC], f32, name="cA", tag="cA")
            cB = npool.tile([P, QC], f32, name="cB", tag="cB")
            nc.vector.tensor_copy(cA[:], nA[:])
            nc.vector.tensor_copy(cB[:], nB[:])
            r2a = rpool.tile([1, QC], f32, name="r2a", tag="r2a")
            r2b = rpool.tile([1, QC], f32, name="r2b", tag="r2b")
            nc.vector.tensor_copy(r2a[:], cA[64:65, :])
            nc.vector.tensor_copy(r2b[:], cB[32:33, :])
            r3a = rpool.tile([1, QC], f32, name="r3a", tag="r3a")
            r3b = rpool.tile([1, QC], f32, name="r3b", tag="r3b")
            nc.vector.reciprocal_approx_fast(r3a[:], r2a[:])
            nc.vector.reciprocal_approx_fast(r3b[:], r2b[:])
            rbc1 = pp.tile([64, QC], f32, name="rbc1", tag="ps_p")
            rbc2 = pp.tile([64, QC], f32, name="rbc2", tag="ps_p")
            nc.tensor.matmul(rbc1[:], ones[0:1, 0:64], r3a[:], start=True, stop=True)
            nc.tensor.matmul(rbc2[:], ones[0:1, 0:64], r3b[:], start=True, stop=True)
            rbc = rpool.tile([P, QC], f32, name="rbc", tag="rbc")
            nc.vector.tensor_copy(rbc[0:64, :], rbc1[:])
            nc.vector.tensor_copy(rbc[64:P, :], rbc2[0:64, :])
            nc.vector.tensor_mul(aot[r][0:64, ds(qc * QC, QC)], cA[0:64, :], rbc[0:64, :])
            nc.vector.tensor_mul(aot[r][64:P, ds(qc * QC, QC)], cB[64:P, :], rbc[64:P, :])

        def qk_units(dst, w, b, r, qc):
            cell = []
            return [lambda: cell.append(proj_qk_a(dst, w, b, r, qc)),
                    lambda: proj_qk_b(cell[0], dst, w, b, r, qc)]

        def v_units(t):
            cell = []
            return [lambda t=t: cell.append(proj_v_a(t)),
                    lambda t=t: proj_v_b(cell[0], t)]

        # startup: KT spans all k tokens, so the first KT-pair0 chain (plus
        # the first QT chunk and the first two V token-tiles) precede
        # attention; everything else is sprinkled into attention slots.
        proj_qk(qt[0], wq, bq, 0, 0)
        proj_qk(kt[0], wk, bk, 0, 0)
        proj_v(0)
        proj_v(1)

        pending_norm = None
        for r in range(NPAIR):
            for qc in range(NQC):
                # unit queue for this (r, qc) window: each unit is ~1us of
                # PE work, drained a few per slot
                units = []
                if r == 0 and qc == 0:
                    units += qk_units(kt[0], wk, bk, 0, 1)
                    units += v_units(2) + v_units(3)
                    units += qk_units(kt[0], wk, bk, 0, 2)
                    units += v_units(4) + v_units(5)
                    units += qk_units(kt[0], wk, bk, 0, 3)
                    for t in range(6, 9):
                        units += v_units(t)
                    units += qk_units(qt[0], wq, bq, 0, 1)
                    for t in range(9, 12):
                        units += v_units(t)
                    units += qk_units(qt[0], wq, bq, 0, 2)
                    for t in range(12, NT):
                        units += v_units(t)
                    units += qk_units(qt[0], wq, bq, 0, 3)
                elif r == 0:
                    # pair-1 projections squeezed into the 3 remaining
                    # pair-0 windows (KT chains first)
                    chains = [("k", 0), ("k", 1), ("k", 2), ("k", 3),
                              ("q", 0), ("q", 1), ("q", 2), ("q", 3)]
                    for kind, c in chains[(qc - 1) * 3:qc * 3]:
                        dst, w_, b_ = ((kt[1], wk, bk) if kind == "k"
                                       else (qt[1], wq, bq))
                        units += qk_units(dst, w_, b_, 1, c)
                elif r < NPAIR - 1:
                    # next pair's projections: KT chains first (needed from
                    # t=0 of its qc0), QT chains later
                    kind = ("k", kt) if qc < 2 else ("q", qt)
                    w_, b_ = (wk, bk) if qc < 2 else (wq, bq)
                    c0 = 2 * qc if qc < 2 else 2 * (qc - 2)
                    units += qk_units(kind[1][r + 1], w_, b_, r + 1, c0)
                    units += qk_units(kind[1][r + 1], w_, b_, r + 1, c0 + 1)
                if r == NPAIR - 1 and qc > 0:
                    for i in range(8):
                        j = (qc - 1) * NPAIR + i // 2
                        units.append(lambda j=j, n=i % 2: proj_out(j, n))

                nA = ps_n.tile([65, QC], f32, name="nA", tag="nA")
                nB = ps_n.tile([P, QC], f32, name="nB", tag="nB")
                es = {}
                ui = 0
                for t in range(NT):
                    sc = ps_s.tile([P, 2, QC], f32, name="sc", tag="sc")
                    nc.tensor.matmul(sc[:, 0, :], kt[r][0:64, ds(t * P, P)],
                                     qt[r][0:64, ds(qc * QC, QC)], start=True, stop=True)
                    nc.tensor.matmul(sc[:, 1, :], kt[r][64:P, ds(t * P, P)],
                                     qt[r][64:P, ds(qc * QC, QC)], start=True, stop=True)
                    e = epool.tile([P, 2, QC], bf16, name="eT", tag="eT")
                    nc.scalar.activation(e[:], sc[:], EXP)
                    es[t] = e
                    if t == 0 and pending_norm is not None:
                        emit_normalize(pending_norm)
                        pending_norm = None
                    if t >= 1:
                        tp = t - 1
                        nc.tensor.matmul(nA[:], vsb[:, tp, r, 0:65], es[tp][:, 0, :],
                                         start=(tp == 0), stop=False)
                        nc.tensor.matmul(nB[:], vsb[:, tp, r, 65:193], es[tp][:, 1, :],
                                         start=(tp == 0), stop=False)
                        del es[tp]
                    # drain the unit queue evenly across remaining slots
                    want = -(-(len(units) - ui) // (NT - t))
                    for _ in range(want):
                        units[ui]()
                        ui += 1
                while ui < len(units):
                    units[ui]()
                    ui += 1
                tp = NT - 1
                nc.tensor.matmul(nA[:], vsb[:, tp, r, 0:65], es[tp][:, 0, :],
                                 start=False, stop=True)
                nc.tensor.matmul(nB[:], vsb[:, tp, r, 65:193], es[tp][:, 1, :],
                                 start=False, stop=True)
                del es[tp]
                pending_norm = (nA, nB, r, qc)

        emit_normalize(pending_norm)
        # tail: output projection of the last q-chunk
        for j in range((NQC - 1) * NPAIR, NT):
            for n in range(D // QC):
                proj_out(j, n)

    nc.compile()

    _CACHED_NC = nc
    return nc


def prepare_in_maps(inputs):
    x = np.asarray(inputs["x"], np.float32)
    Wq = np.asarray(inputs["Wq"], np.float32)
    bq = np.asarray(inputs["bq"], np.float32)
    Wk = np.asarray(inputs["Wk"], np.float32)
    bk = np.asarray(inputs["bk"], np.float32)
    Wv = np.asarray(inputs["Wv"], np.float32)
    Wo = np.asarray(inputs["Wo"], np.float32)
    in_maps = []
    for c in range(8):
        b, half = c // 2, c % 2
        cols = slice(half * DH, (half + 1) * DH)
        in_maps.append({
            "xt": np.ascontiguousarray(x[b].T).astype(BF),
            "wq": np.ascontiguousarray(Wq[:, cols] / 8.0).astype(BF),
            "wk": np.ascontiguousarray(Wk[:, cols]).astype(BF),
            "wv": np.ascontiguousarray(Wv[:, cols]).astype(BF),
            "wo": np.ascontiguousarray(Wo[cols, :]).astype(BF),
            "bq2": np.ascontiguousarray((bq[cols] / 8.0).astype(np.float32).reshape(NPAIR, P).T),
            "bk2": np.ascontiguousarray(bk[cols].astype(np.float32).reshape(NPAIR, P).T),
        })
    return in_maps


def postprocess(results, inputs):
    bv = np.asarray(inputs["bv"], np.float64)
    Wo = np.asarray(inputs["Wo"], np.float64)
    bo = np.asarray(inputs["bo"], np.float64)
    bo_eff = (bv @ Wo + bo).astype(np.float32)
    out = np.empty((4, T, D), np.float32)
    for b in range(4):
        out[b] = (results[2 * b]["out"].astype(np.float32)
                  + results[2 * b + 1]["out"].astype(np.float32)
                  + bo_eff[None, :])
    return out


def kernel(**inputs):
    from concourse.bass_utils import run_bass_kernel_spmd
    nc = build_nc()
    in_maps = prepare_in_maps(inputs)
    res = run_bass_kernel_spmd(nc, in_maps, core_ids=list(range(8)))
    return postprocess(res.results, inputs)
